# revision 17
# baseline (speedup 1.0000x reference)
"""CBAM attention (channel + spatial) Trainium2 Bass kernel.

Full inputs in, full output out. Data-parallel over batch: B=32 samples
split 4-per-core across 8 NeuronCores; params replicated.

Per-core dataflow (per sample, x_b kept resident in SBUF = 8 MiB):
  load x_b -> HW pooling (sum via ACT accum_out, max via DVE reduce)
  -> channel-attention MLP on PE/ACT (spectral-norm scales folded in)
  -> channel max of s*x via DVE stt chain + PE-transpose partition reduce
  -> channel sum of s*x via PE matmuls (s as weights)
  -> 3x3 conv on [64(H),2,64(W)] via 18 fused scalar_tensor_tensor taps
  -> sigmoid -> PE ones-broadcast into PSUM -> out = (x*s)*sig in place
HBM traffic is the minimum: read x once, write out once (64 MiB/core).

The walrus build here allows one sync wait per instruction, no ext-isa
gpsimd ops, no 2-input Pool ops, and no partition-base mismatch between
SBUF operands; waitsplit.split_multi_waits handles the wait limit and
the kernel avoids the rest.
"""
import numpy as np
from contextlib import ExitStack

import concourse.bass as bass
import concourse.mybir as mybir
from concourse.tile import TileContext
from concourse.bass_utils import run_bass_kernel_spmd

F32 = mybir.dt.float32
BF16 = mybir.dt.bfloat16
ALU = mybir.AluOpType
ACTF = mybir.ActivationFunctionType
AX = mybir.AxisListType

B, C, H, W = 32, 512, 64, 64
NCORES = 8
BPC = B // NCORES          # samples per core
HWF = H * W                # 4096
NCH = C // 128             # 4 channel chunks of 128
Cr = C // 8                # 64
EPS = 1e-12

_cache = {}


def _split_multi_waits(nc):
    import bass_rust
    fn = nc.m.functions[0]
    n_split = 0
    uid = 0
    for bb in list(fn.blocks):
        insts = bb.instructions
        out = []
        changed = False
        for ins in insts:
            si = ins.sync_info
            waits = list(si.on_wait) if si and si.on_wait else []
            if len(waits) > 1:
                changed = True
                for w in waits[:-1]:
                    nop = bass_rust.InstNoOp(name=f"wsplit_{uid}")
                    uid += 1
                    nop.engine = ins.engine
                    nop.sync_info = bass_rust.SyncInfo(on_wait=[w], on_update=[])
                    nc.register_instruction(nop, overwrite=True)
                    out.append(nop)
                    n_split += 1
                ins.sync_info = bass_rust.SyncInfo(
                    on_wait=waits[-1:], on_update=list(si.on_update or []))
            out.append(ins)
        if changed:
            bb.instructions = out
    return n_split


def _build_nc():
    nc = bass.Bass("TRN2", debug=False)

    x_ext = nc.declare_dram_parameter("x", [BPC, C, H, W], F32, isOutput=False)
    w1_ext = nc.declare_dram_parameter("w1", [Cr, C], F32, isOutput=False)       # W1m
    w1t_ext = nc.declare_dram_parameter("w1t", [128, NCH, Cr], F32, isOutput=False)  # W1m.T chunks
    b1_ext = nc.declare_dram_parameter("b1", [Cr, 1], F32, isOutput=False)
    u1t_ext = nc.declare_dram_parameter("u1t", [Cr, 1], F32, isOutput=False)
    w2_ext = nc.declare_dram_parameter("w2", [128, NCH, Cr], F32, isOutput=False)    # W2m chunks
    w2t_ext = nc.declare_dram_parameter("w2t", [Cr, NCH, 128], F32, isOutput=False)  # W2m.T chunks
    b2_ext = nc.declare_dram_parameter("b2", [128, NCH], F32, isOutput=False)
    u2t_ext = nc.declare_dram_parameter("u2t", [128, NCH], F32, isOutput=False)
    w3_ext = nc.declare_dram_parameter("w3", [1, 18], F32, isOutput=False)
    b3_ext = nc.declare_dram_parameter("b3", [1, 1], F32, isOutput=False)
    u3_ext = nc.declare_dram_parameter("u3", [1, 1], F32, isOutput=False)
    id_ext = nc.declare_dram_parameter("ident", [128, 128], F32, isOutput=False)
    out_ext = nc.declare_dram_parameter("out", [BPC, C, H, W], F32, isOutput=True)

    # DRAM views: [128 part, chunk, hw]
    x_v = [x_ext[b].rearrange("c h w -> c (h w)").rearrange("(g p) f -> p g f", p=128)
           for b in range(BPC)]
    o_v = [out_ext[b].rearrange("c h w -> c (h w)").rearrange("(g p) f -> p g f", p=128)
           for b in range(BPC)]

    with TileContext(nc) as tc, ExitStack() as ctx:
        const = ctx.enter_context(tc.tile_pool(name="const", bufs=1))
        xpool = ctx.enter_context(tc.tile_pool(name="x", bufs=4))
        mid = ctx.enter_context(tc.tile_pool(name="mid", bufs=2))
        csr = ctx.enter_context(tc.tile_pool(name="csr", bufs=2))
        stats = ctx.enter_context(tc.tile_pool(name="stats", bufs=2))
        convp_ = ctx.enter_context(tc.tile_pool(name="conv", bufs=2))

        # ---------------- setup: load params ----------------
        w1sb = const.tile([Cr, C], F32)
        nc.sync.dma_start(out=w1sb[:], in_=w1_ext[:])
        w1t = const.tile([128, NCH, Cr], F32)
        nc.sync.dma_start(out=w1t[:], in_=w1t_ext[:])
        b1t = const.tile([Cr, 1], F32)
        nc.sync.dma_start(out=b1t[:], in_=b1_ext[:])
        u1t = const.tile([Cr, 1], F32)
        nc.sync.dma_start(out=u1t[:], in_=u1t_ext[:])
        w2sb = const.tile([128, NCH, Cr], F32)
        nc.sync.dma_start(out=w2sb[:], in_=w2_ext[:])
        w2t = const.tile([Cr, NCH, 128], F32)
        nc.sync.dma_start(out=w2t[:], in_=w2t_ext[:])
        b2c = const.tile([128, NCH], F32)
        nc.sync.dma_start(out=b2c[:], in_=b2_ext[:])
        u2t = const.tile([128, NCH], F32)
        nc.sync.dma_start(out=u2t[:], in_=u2t_ext[:])
        w3sb = const.tile([1, 18], F32)
        nc.sync.dma_start(out=w3sb[:], in_=w3_ext[:])
        b3sb = const.tile([1, 1], F32)
        nc.sync.dma_start(out=b3sb[:], in_=b3_ext[:])
        u3sb = const.tile([1, 1], F32)
        nc.sync.dma_start(out=u3sb[:], in_=u3_ext[:])
        ident = const.tile([128, 128], F32)
        nc.sync.dma_start(out=ident[:], in_=id_ext[:])

        ones = const.tile([128, 1], F32)
        nc.vector.memset(ones[:], 1.0)
        onesr = const.tile([1, 128], F32)
        nc.vector.memset(onesr[:], 1.0)
        b2x2 = const.tile([128, NCH], F32)
        nc.vector.tensor_scalar(b2x2[:], b2c[:], 2.0, None, ALU.mult)

        taps64 = const.tile([H, 18], F32)
        b3rep = const.tile([H, 1], F32)
        scal128 = const.tile([128, 3], F32)

        # ---------------- setup: spectral norms ----------------
        with tc.tile_pool(name="pssn", bufs=2, space="PSUM") as pssn, \
             tc.tile_pool(name="snsb", bufs=1) as snsb:
            # --- sv1 for w1 ([Cr, C]); sv = ||q|| / ||r||, r = Wm.T u, q = Wm r ---
            r1t_ps = pssn.tile([128, NCH], F32, tag="big")
            for c_ in range(NCH):
                nc.tensor.matmul(r1t_ps[:, c_:c_ + 1], w1sb[:, c_ * 128:(c_ + 1) * 128],
                                 u1t[:], start=True, stop=True)
            r1t = snsb.tile([128, NCH], F32)
            nc.scalar.copy(r1t[:], r1t_ps[:])
            r1sq = snsb.tile([128, NCH], F32)
            r1sqa = snsb.tile([128, 1], F32)
            nc.scalar.activation(out=r1sq[:], in_=r1t_ps[:], func=ACTF.Square,
                                 accum_out=r1sqa[:])
            n1_ps = pssn.tile([1, 1], F32, tag="dot")
            nc.tensor.matmul(n1_ps[:], r1sqa[:], ones[:], start=True, stop=True)
            n1 = snsb.tile([1, 1], F32)
            nc.scalar.activation(out=n1[:], in_=n1_ps[:], func=ACTF.Sqrt)
            nc.vector.tensor_scalar(n1[:], n1[:], EPS, None, ALU.max)
            rinv1 = snsb.tile([1, 1], F32)
            nc.vector.reciprocal(rinv1[:], n1[:])

            q1_ps = pssn.tile([Cr, 1], F32, tag="big")
            for c_ in range(NCH):
                nc.tensor.matmul(q1_ps[:], w1t[:, c_, :], r1t[:, c_:c_ + 1],
                                 start=(c_ == 0), stop=(c_ == NCH - 1))
            q1sq = snsb.tile([Cr, 1], F32)
            nc.scalar.activation(out=q1sq[:], in_=q1_ps[:], func=ACTF.Square)
            nq1_ps = pssn.tile([1, 1], F32, tag="dot")
            nc.tensor.matmul(nq1_ps[:], q1sq[:], ones[0:Cr, :], start=True, stop=True)
            sv1 = snsb.tile([1, 1], F32)
            nc.scalar.activation(out=sv1[:], in_=nq1_ps[:], func=ACTF.Sqrt)
            nc.vector.tensor_tensor(sv1[:], sv1[:], rinv1[:], ALU.mult)
            inv1 = snsb.tile([1, 1], F32)
            nc.vector.reciprocal(inv1[:], sv1[:])
            csavg = snsb.tile([1, 1], F32)
            nc.vector.tensor_scalar(csavg[:], inv1[:], 1.0 / HWF, None, ALU.mult)

            # --- sv2 for w2 ([C, Cr]) ---
            r2t_ps = pssn.tile([Cr, 1], F32, tag="big")
            for c_ in range(NCH):
                nc.tensor.matmul(r2t_ps[:], w2sb[:, c_, :], u2t[:, c_:c_ + 1],
                                 start=(c_ == 0), stop=(c_ == NCH - 1))
            r2t = snsb.tile([Cr, 1], F32)
            nc.scalar.copy(r2t[:], r2t_ps[:])
            r2sq = snsb.tile([Cr, 1], F32)
            nc.scalar.activation(out=r2sq[:], in_=r2t_ps[:], func=ACTF.Square)
            n2_ps = pssn.tile([1, 1], F32, tag="dot")
            nc.tensor.matmul(n2_ps[:], r2sq[:], ones[0:Cr, :], start=True, stop=True)
            n2 = snsb.tile([1, 1], F32)
            nc.scalar.activation(out=n2[:], in_=n2_ps[:], func=ACTF.Sqrt)
            nc.vector.tensor_scalar(n2[:], n2[:], EPS, None, ALU.max)
            rinv2 = snsb.tile([1, 1], F32)
            nc.vector.reciprocal(rinv2[:], n2[:])

            q2_ps = pssn.tile([128, NCH], F32, tag="big")
            for c_ in range(NCH):
                nc.tensor.matmul(q2_ps[:, c_:c_ + 1], w2t[:, c_, :], r2t[:],
                                 start=True, stop=True)
            q2sq = snsb.tile([128, NCH], F32)
            q2sqa = snsb.tile([128, 1], F32)
            nc.scalar.activation(out=q2sq[:], in_=q2_ps[:], func=ACTF.Square,
                                 accum_out=q2sqa[:])
            nq2_ps = pssn.tile([1, 1], F32, tag="dot")
            nc.tensor.matmul(nq2_ps[:], q2sqa[:], ones[:], start=True, stop=True)
            sv2 = snsb.tile([1, 1], F32)
            nc.scalar.activation(out=sv2[:], in_=nq2_ps[:], func=ACTF.Sqrt)
            nc.vector.tensor_tensor(sv2[:], sv2[:], rinv2[:], ALU.mult)
            inv2 = snsb.tile([1, 1], F32)
            nc.vector.reciprocal(inv2[:], sv2[:])

            # --- sv3 for w3 ([1, 18]) ---
            v3 = snsb.tile([1, 18], F32)
            nc.vector.tensor_scalar(v3[:], w3sb[:], u3sb[0:1, 0:1], None, ALU.mult)
            v3sq = snsb.tile([1, 18], F32)
            n3sq = snsb.tile([1, 1], F32)
            nc.scalar.activation(out=v3sq[:], in_=v3[:], func=ACTF.Square,
                                 accum_out=n3sq[:])
            n3 = snsb.tile([1, 1], F32)
            nc.scalar.activation(out=n3[:], in_=n3sq[:], func=ACTF.Sqrt)
            nc.vector.tensor_scalar(n3[:], n3[:], EPS, None, ALU.max)
            rinv3 = snsb.tile([1, 1], F32)
            nc.vector.reciprocal(rinv3[:], n3[:])
            t3 = snsb.tile([1, 18], F32)
            nc.vector.tensor_tensor(t3[:], v3[:], w3sb[:], ALU.mult)
            t3s = snsb.tile([1, 1], F32)
            nc.vector.tensor_reduce(out=t3s[:], in_=t3[:], axis=AX.X, op=ALU.add)
            nc.vector.tensor_tensor(t3s[:], t3s[:], rinv3[:], ALU.mult)
            sv3 = snsb.tile([1, 1], F32)
            nc.scalar.activation(out=sv3[:], in_=t3s[:], func=ACTF.Abs)
            inv3 = snsb.tile([1, 1], F32)
            nc.vector.reciprocal(inv3[:], sv3[:])

            # --- fold scales: taps (w3/sv3, ch0 also /C), replicated scalars ---
            tapr = snsb.tile([1, 18], F32)
            nc.vector.tensor_scalar(tapr[:], w3sb[:], inv3[0:1, 0:1], None, ALU.mult)
            nc.vector.tensor_scalar(tapr[0:1, 0:9], tapr[0:1, 0:9], 1.0 / C, None,
                                    ALU.mult)
            scalr = snsb.tile([1, 3], F32)
            nc.vector.tensor_copy(scalr[0:1, 0:1], csavg[:])
            nc.vector.tensor_copy(scalr[0:1, 1:2], inv1[:])
            nc.vector.tensor_copy(scalr[0:1, 2:3], inv2[:])

            # replicate across partitions via PE ones-broadcast
            bc_ps = pssn.tile([128, 32], F32, tag="big")
            nc.tensor.matmul(bc_ps[0:H, 0:18], onesr[0:1, 0:H], tapr[:],
                             start=True, stop=True)
            nc.tensor.matmul(bc_ps[0:H, 18:19], onesr[0:1, 0:H], b3sb[:],
                             start=True, stop=True)
            nc.tensor.matmul(bc_ps[:, 19:22], onesr[:], scalr[:],
                             start=True, stop=True)
            nc.scalar.copy(taps64[:], bc_ps[0:H, 0:18])
            nc.scalar.copy(b3rep[:], bc_ps[0:H, 18:19])
            nc.scalar.copy(scal128[:], bc_ps[:, 19:22])

        # main-loop PSUM pools (created after the setup pool is closed so the
        # stack allocator reuses its banks; total = 2+2+2+2 = 8 banks)
        psmlp = ctx.enter_context(tc.tile_pool(name="psmlp", bufs=1, space="PSUM"))
        pscs = ctx.enter_context(tc.tile_pool(name="pscs", bufs=2, space="PSUM"))
        pstp = ctx.enter_context(tc.tile_pool(name="pstp", bufs=2, space="PSUM"))
        psbc = ctx.enter_context(tc.tile_pool(name="psbc", bufs=2, space="PSUM"))
        xb16p = ctx.enter_context(tc.tile_pool(name="xb16", bufs=4))

        # ---------------- main loop over samples ----------------
        for b in range(BPC):
            xt = []
            for hf in range(2):
                xh = xpool.tile([128, 2, HWF], F32, tag="x")
                nc.sync.dma_start(out=xh[:], in_=x_v[b][:, 2 * hf:2 * hf + 2, :])
                xt.append(xh)
            xv = [xt[cc // 2][:, cc % 2, :] for cc in range(NCH)]

            # t: cmax-chain accumulator; also scratch dest for ACT sum-pool
            t = mid.tile([128, HWF], F32, tag="mid")

            avgs = stats.tile([128, NCH], F32, tag="avg")
            mxs = stats.tile([128, NCH], F32, tag="mx")
            for cc in range(NCH):
                nc.scalar.activation(out=t[:], in_=xv[cc], func=ACTF.Copy,
                                     accum_out=avgs[:, cc:cc + 1])
                nc.vector.tensor_reduce(out=mxs[:, cc:cc + 1], in_=xv[cc],
                                        axis=AX.X, op=ALU.max)

            # channel-attention MLP -> s (chunked [128, NCH])
            h_ps = psmlp.tile([Cr, 2], F32, tag="mlp")
            for c_ in range(NCH):
                nc.tensor.matmul(h_ps[:, 0:1], w1t[:, c_, :], avgs[:, c_:c_ + 1],
                                 start=(c_ == 0), stop=(c_ == NCH - 1))
            for c_ in range(NCH):
                nc.tensor.matmul(h_ps[:, 1:2], w1t[:, c_, :], mxs[:, c_:c_ + 1],
                                 start=(c_ == 0), stop=(c_ == NCH - 1))
            h_sb = stats.tile([Cr, 2], F32, tag="hsb")
            nc.scalar.activation(out=h_sb[:, 0:1], in_=h_ps[:, 0:1], func=ACTF.Relu,
                                 bias=b1t[:], scale=scal128[0:Cr, 0:1])
            nc.scalar.activation(out=h_sb[:, 1:2], in_=h_ps[:, 1:2], func=ACTF.Relu,
                                 bias=b1t[:], scale=scal128[0:Cr, 1:2])
            hs = stats.tile([Cr, 1], F32, tag="hs")
            nc.vector.tensor_tensor(hs[:], h_sb[:, 0:1], h_sb[:, 1:2], ALU.add)
            a_ps = psmlp.tile([128, NCH], F32, tag="mlp")
            for c_ in range(NCH):
                nc.tensor.matmul(a_ps[:, c_:c_ + 1], w2t[:, c_, :], hs[:],
                                 start=True, stop=True)
            s_t = stats.tile([128, NCH], F32, tag="st")
            for c_ in range(NCH):
                nc.scalar.activation(out=s_t[:, c_:c_ + 1], in_=a_ps[:, c_:c_ + 1],
                                     func=ACTF.Sigmoid, bias=b2x2[:, c_:c_ + 1],
                                     scale=scal128[:, 2:3])

            # channel max of s*x: ACT init + DVE stt chain -> t
            nc.scalar.activation(out=t[:], in_=xv[0], func=ACTF.Copy,
                                 scale=s_t[:, 0:1])
            for cc in range(1, NCH):
                nc.vector.scalar_tensor_tensor(out=t[:], in0=xv[cc],
                                               scalar=s_t[:, cc:cc + 1], in1=t[:],
                                               op0=ALU.mult, op1=ALU.max)
            # partition reduce via PE transposes: cmaxT[p, m] = max_c t[c, 128m+p]
            cmaxT = stats.tile([128, 32], F32, tag="cmaxT")
            for g in range(8):
                ps = pstp.tile([128, 4, 128], F32, tag="tp")
                for mm in range(4):
                    m = 4 * g + mm
                    nc.tensor.transpose(ps[:, mm, :], t[:, 128 * m:128 * (m + 1)],
                                        ident[:])
                nc.vector.tensor_reduce(out=cmaxT[:, 4 * g:4 * (g + 1)], in_=ps[:],
                                        axis=AX.X, op=ALU.max)
            t2 = pstp.tile([32, 128], F32, tag="tp")
            nc.tensor.transpose(t2[:], cmaxT[:], ident[:])
            t2sb = convp_.tile([32, 128], F32, tag="t2sb")
            nc.scalar.copy(t2sb[:], t2[:])
            conv_in = convp_.tile([H, 2, W], F32, tag="cin")
            nc.sync.dma_start(out=conv_in[:, 1, :],
                              in_=t2sb[:].rearrange("q (r w) -> q r w", r=2))

            # channel sum of s*x via PE in bf16 (stat path tolerates bf16;
            # gpsimd converts x chunks just-in-time), evac + reshape per chunk
            s16 = stats.tile([128, NCH], BF16, tag="s16")
            nc.gpsimd.tensor_copy(out=s16[:], in_=s_t[:])
            RPC = 512 // W
            for j in range(HWF // 512):
                ps = pscs.tile([1, 512], F32, tag="cs")
                for c_ in range(NCH):
                    xb = xb16p.tile([128, 512], BF16, tag="xb")
                    nc.gpsimd.tensor_copy(out=xb[:], in_=xv[c_][:, 512 * j:512 * (j + 1)])
                    nc.tensor.matmul(ps[:], s16[:, c_:c_ + 1], xb[:],
                                     start=(c_ == 0), stop=(c_ == NCH - 1))
                crow = csr.tile([1, 512], F32, tag="csrow")
                nc.scalar.copy(crow[:], ps[:])
                nc.sync.dma_start(
                    out=conv_in[RPC * j:RPC * (j + 1), 0, :],
                    in_=crow[0:1, :].rearrange("p (h w) -> p h w", h=RPC))

            # 3x3 conv, zero pad: H-shifts via DMA copies, W-shifts via free offsets
            convm = convp_.tile([H, 2, W], F32, tag="cm")   # [r] = conv_in[r+1]
            convp = convp_.tile([H, 2, W], F32, tag="cp")   # [r] = conv_in[r-1]
            nc.gpsimd.memset(convm[:], 0.0)
            nc.gpsimd.memset(convp[:], 0.0)
            nc.sync.dma_start(out=convm[0:H - 1, :, :], in_=conv_in[1:H, :, :])
            nc.sync.dma_start(out=convp[1:H, :, :], in_=conv_in[0:H - 1, :, :])
            y64 = convp_.tile([H, W], F32, tag="y")
            srcs = {0: convp, 1: conv_in, 2: convm}
            nc.vector.tensor_scalar(y64[:], conv_in[:, 0, :], taps64[:, 4:5], None,
                                    ALU.mult)
            for ch in range(2):
                for kh in range(3):
                    src = srcs[kh]
                    for kw in range(3):
                        j = ch * 9 + kh * 3 + kw
                        if j == 4:
                            continue
                        if kw == 1:
                            o_sl, i_sl = slice(0, W), slice(0, W)
                        elif kw == 2:
                            o_sl, i_sl = slice(0, W - 1), slice(1, W)
                        else:
                            o_sl, i_sl = slice(1, W), slice(0, W - 1)
                        nc.vector.scalar_tensor_tensor(
                            out=y64[:, o_sl], in0=src[:, ch, i_sl],
                            scalar=taps64[:, j:j + 1], in1=y64[:, o_sl],
                            op0=ALU.mult, op1=ALU.add)
            nc.scalar.activation(out=y64[:], in_=y64[:], func=ACTF.Sigmoid,
                                 bias=b3rep[:])

            # sigmoid row -> PE broadcast to PSUM -> ACT evac to full-width SBUF
            # sigB -> 4 wide fused final multiplies on DVE
            sigv = mid.tile([1, HWF], F32, tag="mid")
            nc.sync.dma_start(out=sigv[0:1, :].rearrange("p (h w) -> p h w", h=H),
                              in_=y64[:])
            sigB = mid.tile([128, HWF], F32, tag="mid")
            for j in range(HWF // 512):
                pb = psbc.tile([128, 512], F32, tag="bc")
                nc.tensor.matmul(pb[:], onesr[:], sigv[0:1, 512 * j:512 * (j + 1)],
                                 start=True, stop=True)
                nc.scalar.copy(sigB[:, 512 * j:512 * (j + 1)], pb[:])
            for cc in range(NCH):
                nc.vector.scalar_tensor_tensor(
                    out=xv[cc], in0=xv[cc],
                    scalar=s_t[:, cc:cc + 1], in1=sigB[:],
                    op0=ALU.mult, op1=ALU.mult)
            for hf in range(2):
                nc.sync.dma_start(out=o_v[b][:, 2 * hf:2 * hf + 2, :], in_=xt[hf][:])

    _split_multi_waits(nc)
    return nc


def _get_nc():
    if "nc" not in _cache:
        _cache["nc"] = _build_nc()
    return _cache["nc"]


def _prep_in_maps(inputs):
    f = lambda a: np.ascontiguousarray(np.asarray(a, dtype=np.float32))
    x = f(inputs["x"])
    w1 = f(inputs["w1"]).reshape(Cr, C)
    w2 = f(inputs["w2"]).reshape(C, Cr)
    w3 = f(inputs["w3"]).reshape(1, 18)
    common = {
        "w1": w1,
        "w1t": np.ascontiguousarray(w1.T.reshape(NCH, 128, Cr).transpose(1, 0, 2)),
        "b1": f(inputs["b1"]).reshape(Cr, 1),
        "u1t": f(inputs["u1"]).reshape(Cr, 1),
        "w2": np.ascontiguousarray(w2.reshape(NCH, 128, Cr).transpose(1, 0, 2)),
        "w2t": np.ascontiguousarray(w2.T.reshape(Cr, NCH, 128)),
        "b2": np.ascontiguousarray(f(inputs["b2"]).reshape(NCH, 128).T),
        "u2t": np.ascontiguousarray(f(inputs["u2"]).reshape(NCH, 128).T),
        "w3": w3,
        "b3": f(inputs["b3"]).reshape(1, 1),
        "u3": f(inputs["u3"]).reshape(1, 1),
        "ident": np.eye(128, dtype=np.float32),
    }
    return [dict(common, x=np.ascontiguousarray(x[k * BPC:(k + 1) * BPC]))
            for k in range(NCORES)]


def run(inputs, trace=False, **kw):
    nc = _get_nc()
    in_maps = _prep_in_maps(inputs)
    res = run_bass_kernel_spmd(nc, in_maps, list(range(NCORES)), trace=trace, **kw)
    out = np.concatenate([res.results[k]["out"] for k in range(NCORES)], axis=0)
    return out, res


def kernel(**inputs) -> np.ndarray:
    out, _ = run(inputs)
    return out


# revision 19
# speedup vs baseline: 1.3425x; 1.3425x over previous
"""CBAM attention (channel + spatial) Trainium2 Bass kernel.

Full inputs in, full output out. Data-parallel over batch: B=32 samples
split 4-per-core across 8 NeuronCores; params replicated.

Per-core dataflow (per sample, x_b kept resident in SBUF = 8 MiB):
  load x_b -> HW pooling (sum via ACT accum_out, max via DVE reduce)
  -> channel-attention MLP on PE/ACT (spectral-norm scales folded in)
  -> channel max of s*x via DVE stt chain + PE-transpose partition reduce
  -> channel sum of s*x via PE matmuls (s as weights)
  -> 3x3 conv on [64(H),2,64(W)] via 18 fused scalar_tensor_tensor taps
  -> sigmoid -> PE ones-broadcast into PSUM -> out = (x*s)*sig in place
HBM traffic is the minimum: read x once, write out once (64 MiB/core).

The walrus build here allows one sync wait per instruction, no ext-isa
gpsimd ops, no 2-input Pool ops, and no partition-base mismatch between
SBUF operands; waitsplit.split_multi_waits handles the wait limit and
the kernel avoids the rest.
"""
import numpy as np
from contextlib import ExitStack

import concourse.bass as bass
import concourse.mybir as mybir
from concourse.tile import TileContext
from concourse.bass_utils import run_bass_kernel_spmd

F32 = mybir.dt.float32
BF16 = mybir.dt.bfloat16
ALU = mybir.AluOpType
ACTF = mybir.ActivationFunctionType
AX = mybir.AxisListType

B, C, H, W = 32, 512, 64, 64
NCORES = 8
BPC = B // NCORES          # samples per core
HWF = H * W                # 4096
NCH = C // 128             # 4 channel chunks of 128
Cr = C // 8                # 64
EPS = 1e-12

_cache = {}


def _split_multi_waits(nc):
    import bass_rust
    fn = nc.m.functions[0]
    n_split = 0
    uid = 0
    for bb in list(fn.blocks):
        insts = bb.instructions
        out = []
        changed = False
        for ins in insts:
            si = ins.sync_info
            waits = list(si.on_wait) if si and si.on_wait else []
            if len(waits) > 1:
                changed = True
                for w in waits[:-1]:
                    nop = bass_rust.InstNoOp(name=f"wsplit_{uid}")
                    uid += 1
                    nop.engine = ins.engine
                    nop.sync_info = bass_rust.SyncInfo(on_wait=[w], on_update=[])
                    nc.register_instruction(nop, overwrite=True)
                    out.append(nop)
                    n_split += 1
                ins.sync_info = bass_rust.SyncInfo(
                    on_wait=waits[-1:], on_update=list(si.on_update or []))
            out.append(ins)
        if changed:
            bb.instructions = out
    return n_split


def _build_nc():
    nc = bass.Bass("TRN2", debug=False)

    x_ext = nc.declare_dram_parameter("x", [BPC, C, H, W], F32, isOutput=False)
    w1_ext = nc.declare_dram_parameter("w1", [Cr, C], F32, isOutput=False)       # W1m
    w1t_ext = nc.declare_dram_parameter("w1t", [128, NCH, Cr], F32, isOutput=False)  # W1m.T chunks
    b1_ext = nc.declare_dram_parameter("b1", [Cr, 1], F32, isOutput=False)
    u1t_ext = nc.declare_dram_parameter("u1t", [Cr, 1], F32, isOutput=False)
    w2_ext = nc.declare_dram_parameter("w2", [128, NCH, Cr], F32, isOutput=False)    # W2m chunks
    w2t_ext = nc.declare_dram_parameter("w2t", [Cr, NCH, 128], F32, isOutput=False)  # W2m.T chunks
    b2_ext = nc.declare_dram_parameter("b2", [128, NCH], F32, isOutput=False)
    u2t_ext = nc.declare_dram_parameter("u2t", [128, NCH], F32, isOutput=False)
    w3_ext = nc.declare_dram_parameter("w3", [1, 18], F32, isOutput=False)
    b3_ext = nc.declare_dram_parameter("b3", [1, 1], F32, isOutput=False)
    u3_ext = nc.declare_dram_parameter("u3", [1, 1], F32, isOutput=False)
    id_ext = nc.declare_dram_parameter("ident", [128, 128], F32, isOutput=False)
    out_ext = nc.declare_dram_parameter("out", [BPC, C, H, W], F32, isOutput=True)
    sig_scr = nc.dram_tensor("sig_scratch", [BPC, H, W], F32)

    # DRAM views: [128 part, chunk, hw]
    x_v = [x_ext[b].rearrange("c h w -> c (h w)").rearrange("(g p) f -> p g f", p=128)
           for b in range(BPC)]
    o_v = [out_ext[b].rearrange("c h w -> c (h w)").rearrange("(g p) f -> p g f", p=128)
           for b in range(BPC)]

    with TileContext(nc) as tc, ExitStack() as ctx:
        const = ctx.enter_context(tc.tile_pool(name="const", bufs=1))
        xpool = ctx.enter_context(tc.tile_pool(name="x", bufs=4))
        mid = ctx.enter_context(tc.tile_pool(name="mid", bufs=2))
        csr = ctx.enter_context(tc.tile_pool(name="csr", bufs=2))
        stats = ctx.enter_context(tc.tile_pool(name="stats", bufs=2))
        convp_ = ctx.enter_context(tc.tile_pool(name="conv", bufs=2))

        # ---------------- setup: load params ----------------
        w1sb = const.tile([Cr, C], F32)
        nc.sync.dma_start(out=w1sb[:], in_=w1_ext[:])
        w1t = const.tile([128, NCH, Cr], F32)
        nc.sync.dma_start(out=w1t[:], in_=w1t_ext[:])
        b1t = const.tile([Cr, 1], F32)
        nc.sync.dma_start(out=b1t[:], in_=b1_ext[:])
        u1t = const.tile([Cr, 1], F32)
        nc.sync.dma_start(out=u1t[:], in_=u1t_ext[:])
        w2sb = const.tile([128, NCH, Cr], F32)
        nc.sync.dma_start(out=w2sb[:], in_=w2_ext[:])
        w2t = const.tile([Cr, NCH, 128], F32)
        nc.sync.dma_start(out=w2t[:], in_=w2t_ext[:])
        b2c = const.tile([128, NCH], F32)
        nc.sync.dma_start(out=b2c[:], in_=b2_ext[:])
        u2t = const.tile([128, NCH], F32)
        nc.sync.dma_start(out=u2t[:], in_=u2t_ext[:])
        w3sb = const.tile([1, 18], F32)
        nc.sync.dma_start(out=w3sb[:], in_=w3_ext[:])
        b3sb = const.tile([1, 1], F32)
        nc.sync.dma_start(out=b3sb[:], in_=b3_ext[:])
        u3sb = const.tile([1, 1], F32)
        nc.sync.dma_start(out=u3sb[:], in_=u3_ext[:])
        ident = const.tile([128, 128], F32)
        nc.sync.dma_start(out=ident[:], in_=id_ext[:])

        ones = const.tile([128, 1], F32)
        nc.vector.memset(ones[:], 1.0)
        onesr = const.tile([1, 128], F32)
        nc.vector.memset(onesr[:], 1.0)
        b2x2 = const.tile([128, NCH], F32)
        nc.vector.tensor_scalar(b2x2[:], b2c[:], 2.0, None, ALU.mult)

        taps64 = const.tile([H, 18], F32)
        b3rep = const.tile([H, 1], F32)
        scal128 = const.tile([128, 3], F32)

        # ---------------- setup: spectral norms ----------------
        with tc.tile_pool(name="pssn", bufs=2, space="PSUM") as pssn, \
             tc.tile_pool(name="snsb", bufs=1) as snsb:
            # --- sv1 for w1 ([Cr, C]); sv = ||q|| / ||r||, r = Wm.T u, q = Wm r ---
            r1t_ps = pssn.tile([128, NCH], F32, tag="big")
            for c_ in range(NCH):
                nc.tensor.matmul(r1t_ps[:, c_:c_ + 1], w1sb[:, c_ * 128:(c_ + 1) * 128],
                                 u1t[:], start=True, stop=True)
            r1t = snsb.tile([128, NCH], F32)
            nc.scalar.copy(r1t[:], r1t_ps[:])
            r1sq = snsb.tile([128, NCH], F32)
            r1sqa = snsb.tile([128, 1], F32)
            nc.scalar.activation(out=r1sq[:], in_=r1t_ps[:], func=ACTF.Square,
                                 accum_out=r1sqa[:])
            n1_ps = pssn.tile([1, 1], F32, tag="dot")
            nc.tensor.matmul(n1_ps[:], r1sqa[:], ones[:], start=True, stop=True)
            n1 = snsb.tile([1, 1], F32)
            nc.scalar.activation(out=n1[:], in_=n1_ps[:], func=ACTF.Sqrt)
            nc.vector.tensor_scalar(n1[:], n1[:], EPS, None, ALU.max)
            rinv1 = snsb.tile([1, 1], F32)
            nc.vector.reciprocal(rinv1[:], n1[:])

            q1_ps = pssn.tile([Cr, 1], F32, tag="big")
            for c_ in range(NCH):
                nc.tensor.matmul(q1_ps[:], w1t[:, c_, :], r1t[:, c_:c_ + 1],
                                 start=(c_ == 0), stop=(c_ == NCH - 1))
            q1sq = snsb.tile([Cr, 1], F32)
            nc.scalar.activation(out=q1sq[:], in_=q1_ps[:], func=ACTF.Square)
            nq1_ps = pssn.tile([1, 1], F32, tag="dot")
            nc.tensor.matmul(nq1_ps[:], q1sq[:], ones[0:Cr, :], start=True, stop=True)
            sv1 = snsb.tile([1, 1], F32)
            nc.scalar.activation(out=sv1[:], in_=nq1_ps[:], func=ACTF.Sqrt)
            nc.vector.tensor_tensor(sv1[:], sv1[:], rinv1[:], ALU.mult)
            inv1 = snsb.tile([1, 1], F32)
            nc.vector.reciprocal(inv1[:], sv1[:])
            csavg = snsb.tile([1, 1], F32)
            nc.vector.tensor_scalar(csavg[:], inv1[:], 1.0 / HWF, None, ALU.mult)

            # --- sv2 for w2 ([C, Cr]) ---
            r2t_ps = pssn.tile([Cr, 1], F32, tag="big")
            for c_ in range(NCH):
                nc.tensor.matmul(r2t_ps[:], w2sb[:, c_, :], u2t[:, c_:c_ + 1],
                                 start=(c_ == 0), stop=(c_ == NCH - 1))
            r2t = snsb.tile([Cr, 1], F32)
            nc.scalar.copy(r2t[:], r2t_ps[:])
            r2sq = snsb.tile([Cr, 1], F32)
            nc.scalar.activation(out=r2sq[:], in_=r2t_ps[:], func=ACTF.Square)
            n2_ps = pssn.tile([1, 1], F32, tag="dot")
            nc.tensor.matmul(n2_ps[:], r2sq[:], ones[0:Cr, :], start=True, stop=True)
            n2 = snsb.tile([1, 1], F32)
            nc.scalar.activation(out=n2[:], in_=n2_ps[:], func=ACTF.Sqrt)
            nc.vector.tensor_scalar(n2[:], n2[:], EPS, None, ALU.max)
            rinv2 = snsb.tile([1, 1], F32)
            nc.vector.reciprocal(rinv2[:], n2[:])

            q2_ps = pssn.tile([128, NCH], F32, tag="big")
            for c_ in range(NCH):
                nc.tensor.matmul(q2_ps[:, c_:c_ + 1], w2t[:, c_, :], r2t[:],
                                 start=True, stop=True)
            q2sq = snsb.tile([128, NCH], F32)
            q2sqa = snsb.tile([128, 1], F32)
            nc.scalar.activation(out=q2sq[:], in_=q2_ps[:], func=ACTF.Square,
                                 accum_out=q2sqa[:])
            nq2_ps = pssn.tile([1, 1], F32, tag="dot")
            nc.tensor.matmul(nq2_ps[:], q2sqa[:], ones[:], start=True, stop=True)
            sv2 = snsb.tile([1, 1], F32)
            nc.scalar.activation(out=sv2[:], in_=nq2_ps[:], func=ACTF.Sqrt)
            nc.vector.tensor_tensor(sv2[:], sv2[:], rinv2[:], ALU.mult)
            inv2 = snsb.tile([1, 1], F32)
            nc.vector.reciprocal(inv2[:], sv2[:])

            # --- sv3 for w3 ([1, 18]) ---
            v3 = snsb.tile([1, 18], F32)
            nc.vector.tensor_scalar(v3[:], w3sb[:], u3sb[0:1, 0:1], None, ALU.mult)
            v3sq = snsb.tile([1, 18], F32)
            n3sq = snsb.tile([1, 1], F32)
            nc.scalar.activation(out=v3sq[:], in_=v3[:], func=ACTF.Square,
                                 accum_out=n3sq[:])
            n3 = snsb.tile([1, 1], F32)
            nc.scalar.activation(out=n3[:], in_=n3sq[:], func=ACTF.Sqrt)
            nc.vector.tensor_scalar(n3[:], n3[:], EPS, None, ALU.max)
            rinv3 = snsb.tile([1, 1], F32)
            nc.vector.reciprocal(rinv3[:], n3[:])
            t3 = snsb.tile([1, 18], F32)
            nc.vector.tensor_tensor(t3[:], v3[:], w3sb[:], ALU.mult)
            t3s = snsb.tile([1, 1], F32)
            nc.vector.tensor_reduce(out=t3s[:], in_=t3[:], axis=AX.X, op=ALU.add)
            nc.vector.tensor_tensor(t3s[:], t3s[:], rinv3[:], ALU.mult)
            sv3 = snsb.tile([1, 1], F32)
            nc.scalar.activation(out=sv3[:], in_=t3s[:], func=ACTF.Abs)
            inv3 = snsb.tile([1, 1], F32)
            nc.vector.reciprocal(inv3[:], sv3[:])

            # --- fold scales: taps (w3/sv3, ch0 also /C), replicated scalars ---
            tapr = snsb.tile([1, 18], F32)
            nc.vector.tensor_scalar(tapr[:], w3sb[:], inv3[0:1, 0:1], None, ALU.mult)
            nc.vector.tensor_scalar(tapr[0:1, 0:9], tapr[0:1, 0:9], 1.0 / C, None,
                                    ALU.mult)
            scalr = snsb.tile([1, 3], F32)
            nc.vector.tensor_copy(scalr[0:1, 0:1], csavg[:])
            nc.vector.tensor_copy(scalr[0:1, 1:2], inv1[:])
            nc.vector.tensor_copy(scalr[0:1, 2:3], inv2[:])

            # replicate across partitions via PE ones-broadcast
            bc_ps = pssn.tile([128, 32], F32, tag="big")
            nc.tensor.matmul(bc_ps[0:H, 0:18], onesr[0:1, 0:H], tapr[:],
                             start=True, stop=True)
            nc.tensor.matmul(bc_ps[0:H, 18:19], onesr[0:1, 0:H], b3sb[:],
                             start=True, stop=True)
            nc.tensor.matmul(bc_ps[:, 19:22], onesr[:], scalr[:],
                             start=True, stop=True)
            nc.scalar.copy(taps64[:], bc_ps[0:H, 0:18])
            nc.scalar.copy(b3rep[:], bc_ps[0:H, 18:19])
            nc.scalar.copy(scal128[:], bc_ps[:, 19:22])

        # main-loop PSUM pools (created after the setup pool is closed so the
        # stack allocator reuses its banks; total = 2+2+2+2 = 8 banks)
        psmlp = ctx.enter_context(tc.tile_pool(name="psmlp", bufs=1, space="PSUM"))
        pscs = ctx.enter_context(tc.tile_pool(name="pscs", bufs=1, space="PSUM"))
        pstp = ctx.enter_context(tc.tile_pool(name="pstp", bufs=2, space="PSUM"))
        xb16p = ctx.enter_context(tc.tile_pool(name="xb16", bufs=2))

        # ---------------- main loop over samples ----------------
        for b in range(BPC):
            xt = []
            for hf in range(2):
                xh = xpool.tile([128, 2, HWF], F32, tag="x")
                nc.sync.dma_start(out=xh[:], in_=x_v[b][:, 2 * hf:2 * hf + 2, :])
                xt.append(xh)
            xv = [xt[cc // 2][:, cc % 2, :] for cc in range(NCH)]

            # t: cmax-chain accumulator; also scratch dest for ACT sum-pool
            t = mid.tile([128, HWF], F32, tag="mid")

            avgs = stats.tile([128, NCH], F32, tag="avg")
            mxs = stats.tile([128, NCH], F32, tag="mx")
            for cc in range(NCH):
                nc.scalar.activation(out=t[:], in_=xv[cc], func=ACTF.Copy,
                                     accum_out=avgs[:, cc:cc + 1])
                nc.vector.tensor_reduce(out=mxs[:, cc:cc + 1], in_=xv[cc],
                                        axis=AX.X, op=ALU.max)

            # channel-attention MLP -> s (chunked [128, NCH])
            h_ps = psmlp.tile([Cr, 2], F32, tag="mlp")
            for c_ in range(NCH):
                nc.tensor.matmul(h_ps[:, 0:1], w1t[:, c_, :], avgs[:, c_:c_ + 1],
                                 start=(c_ == 0), stop=(c_ == NCH - 1))
            for c_ in range(NCH):
                nc.tensor.matmul(h_ps[:, 1:2], w1t[:, c_, :], mxs[:, c_:c_ + 1],
                                 start=(c_ == 0), stop=(c_ == NCH - 1))
            h_sb = stats.tile([Cr, 2], F32, tag="hsb")
            nc.scalar.activation(out=h_sb[:, 0:1], in_=h_ps[:, 0:1], func=ACTF.Relu,
                                 bias=b1t[:], scale=scal128[0:Cr, 0:1])
            nc.scalar.activation(out=h_sb[:, 1:2], in_=h_ps[:, 1:2], func=ACTF.Relu,
                                 bias=b1t[:], scale=scal128[0:Cr, 1:2])
            hs = stats.tile([Cr, 1], F32, tag="hs")
            nc.vector.tensor_tensor(hs[:], h_sb[:, 0:1], h_sb[:, 1:2], ALU.add)
            a_ps = psmlp.tile([128, NCH], F32, tag="mlp")
            for c_ in range(NCH):
                nc.tensor.matmul(a_ps[:, c_:c_ + 1], w2t[:, c_, :], hs[:],
                                 start=True, stop=True)
            s_t = stats.tile([128, NCH], F32, tag="st")
            for c_ in range(NCH):
                nc.scalar.activation(out=s_t[:, c_:c_ + 1], in_=a_ps[:, c_:c_ + 1],
                                     func=ACTF.Sigmoid, bias=b2x2[:, c_:c_ + 1],
                                     scale=scal128[:, 2:3])

            # channel max of s*x: ACT init + DVE stt chain -> t
            nc.scalar.activation(out=t[:], in_=xv[0], func=ACTF.Copy,
                                 scale=s_t[:, 0:1])
            for cc in range(1, NCH):
                nc.vector.scalar_tensor_tensor(out=t[:], in0=xv[cc],
                                               scalar=s_t[:, cc:cc + 1], in1=t[:],
                                               op0=ALU.mult, op1=ALU.max)
            # partition reduce via PE transposes: cmaxT[p, m] = max_c t[c, 128m+p]
            cmaxT = stats.tile([128, 32], F32, tag="cmaxT")
            for g in range(8):
                ps = pstp.tile([128, 4, 128], F32, tag="tp")
                for mm in range(4):
                    m = 4 * g + mm
                    nc.tensor.transpose(ps[:, mm, :], t[:, 128 * m:128 * (m + 1)],
                                        ident[:])
                nc.vector.tensor_reduce(out=cmaxT[:, 4 * g:4 * (g + 1)], in_=ps[:],
                                        axis=AX.X, op=ALU.max)
            t2 = pstp.tile([32, 128], F32, tag="tp")
            nc.tensor.transpose(t2[:], cmaxT[:], ident[:])
            t2sb = convp_.tile([32, 128], F32, tag="t2sb")
            nc.scalar.copy(t2sb[:], t2[:])
            conv_in = convp_.tile([H, 2, W], F32, tag="cin")
            nc.sync.dma_start(out=conv_in[:, 1, :],
                              in_=t2sb[:].rearrange("q (r w) -> q r w", r=2))

            # channel sum of s*x via PE in bf16 (stat path tolerates bf16;
            # gpsimd converts x chunks just-in-time), evac + reshape per chunk
            s16 = stats.tile([128, NCH], BF16, tag="s16")
            nc.vector.tensor_copy(s16[:], s_t[:])
            CSW = 2048
            RPC = CSW // W
            for jh in range(HWF // CSW):
                ps = pscs.tile([1, CSW], F32, tag="cs")
                for c_ in range(NCH):
                    xb = xb16p.tile([128, CSW], BF16, tag="xb")
                    nc.vector.tensor_copy(xb[:], xv[c_][:, CSW * jh:CSW * (jh + 1)])
                    for q in range(CSW // 512):
                        nc.tensor.matmul(ps[0:1, 512 * q:512 * (q + 1)],
                                         s16[:, c_:c_ + 1], xb[:, 512 * q:512 * (q + 1)],
                                         start=(c_ == 0), stop=(c_ == NCH - 1))
                crow = csr.tile([1, CSW], F32, tag="csrow")
                nc.scalar.copy(crow[:], ps[:])
                nc.sync.dma_start(
                    out=conv_in[RPC * jh:RPC * (jh + 1), 0, :],
                    in_=crow[0:1, :].rearrange("p (h w) -> p h w", h=RPC))

            # 3x3 conv, zero pad: H-shifts via DMA copies, W-shifts via free offsets
            convm = convp_.tile([H, 2, W], F32, tag="cm")   # [r] = conv_in[r+1]
            convp = convp_.tile([H, 2, W], F32, tag="cp")   # [r] = conv_in[r-1]
            nc.gpsimd.memset(convm[:], 0.0)
            nc.gpsimd.memset(convp[:], 0.0)
            nc.sync.dma_start(out=convm[0:H - 1, :, :], in_=conv_in[1:H, :, :])
            nc.sync.dma_start(out=convp[1:H, :, :], in_=conv_in[0:H - 1, :, :])
            y64 = convp_.tile([H, W], F32, tag="y")
            srcs = {0: convp, 1: conv_in, 2: convm}
            nc.vector.tensor_scalar(y64[:], conv_in[:, 0, :], taps64[:, 4:5], None,
                                    ALU.mult)
            for ch in range(2):
                for kh in range(3):
                    src = srcs[kh]
                    for kw in range(3):
                        j = ch * 9 + kh * 3 + kw
                        if j == 4:
                            continue
                        if kw == 1:
                            o_sl, i_sl = slice(0, W), slice(0, W)
                        elif kw == 2:
                            o_sl, i_sl = slice(0, W - 1), slice(1, W)
                        else:
                            o_sl, i_sl = slice(1, W), slice(0, W - 1)
                        nc.vector.scalar_tensor_tensor(
                            out=y64[:, o_sl], in0=src[:, ch, i_sl],
                            scalar=taps64[:, j:j + 1], in1=y64[:, o_sl],
                            op0=ALU.mult, op1=ALU.add)
            nc.scalar.activation(out=y64[:], in_=y64[:], func=ACTF.Sigmoid,
                                 bias=b3rep[:])

            # sigmoid broadcast over channels via DRAM bounce:
            # y64 -> DRAM scratch -> partition-broadcast read -> sigB
            nc.sync.dma_start(out=sig_scr[b], in_=y64[:])
            flat = sig_scr[b].rearrange("h w -> (h w)")
            bcast_ap = bass.AP(tensor=flat.tensor, offset=flat.offset,
                               ap=[[0, 128]] + list(flat.ap))
            sigB = mid.tile([128, HWF], F32, tag="mid")
            nc.sync.dma_start(out=sigB[:], in_=bcast_ap)
            for cc in range(NCH):
                nc.vector.scalar_tensor_tensor(
                    out=xv[cc], in0=xv[cc],
                    scalar=s_t[:, cc:cc + 1], in1=sigB[:],
                    op0=ALU.mult, op1=ALU.mult)
            for hf in range(2):
                nc.sync.dma_start(out=o_v[b][:, 2 * hf:2 * hf + 2, :], in_=xt[hf][:])

    _split_multi_waits(nc)
    return nc


def _get_nc():
    if "nc" not in _cache:
        _cache["nc"] = _build_nc()
    return _cache["nc"]


def _prep_in_maps(inputs):
    f = lambda a: np.ascontiguousarray(np.asarray(a, dtype=np.float32))
    x = f(inputs["x"])
    w1 = f(inputs["w1"]).reshape(Cr, C)
    w2 = f(inputs["w2"]).reshape(C, Cr)
    w3 = f(inputs["w3"]).reshape(1, 18)
    common = {
        "w1": w1,
        "w1t": np.ascontiguousarray(w1.T.reshape(NCH, 128, Cr).transpose(1, 0, 2)),
        "b1": f(inputs["b1"]).reshape(Cr, 1),
        "u1t": f(inputs["u1"]).reshape(Cr, 1),
        "w2": np.ascontiguousarray(w2.reshape(NCH, 128, Cr).transpose(1, 0, 2)),
        "w2t": np.ascontiguousarray(w2.T.reshape(Cr, NCH, 128)),
        "b2": np.ascontiguousarray(f(inputs["b2"]).reshape(NCH, 128).T),
        "u2t": np.ascontiguousarray(f(inputs["u2"]).reshape(NCH, 128).T),
        "w3": w3,
        "b3": f(inputs["b3"]).reshape(1, 1),
        "u3": f(inputs["u3"]).reshape(1, 1),
        "ident": np.eye(128, dtype=np.float32),
    }
    return [dict(common, x=np.ascontiguousarray(x[k * BPC:(k + 1) * BPC]))
            for k in range(NCORES)]


def run(inputs, trace=False, **kw):
    nc = _get_nc()
    in_maps = _prep_in_maps(inputs)
    res = run_bass_kernel_spmd(nc, in_maps, list(range(NCORES)), trace=trace, **kw)
    out = np.concatenate([res.results[k]["out"] for k in range(NCORES)], axis=0)
    return out, res


def kernel(**inputs) -> np.ndarray:
    out, _ = run(inputs)
    return out


# revision 20
# speedup vs baseline: 1.3737x; 1.0232x over previous
"""CBAM attention (channel + spatial) Trainium2 Bass kernel.

Full inputs in, full output out. Data-parallel over batch: B=32 samples
split 4-per-core across 8 NeuronCores; params replicated.

Per-core dataflow (per sample, x_b kept resident in SBUF = 8 MiB):
  load x_b -> HW pooling (sum via ACT accum_out, max via DVE reduce)
  -> channel-attention MLP on PE/ACT (spectral-norm scales folded in)
  -> channel max of s*x via DVE stt chain + PE-transpose partition reduce
  -> channel sum of s*x via PE matmuls (s as weights)
  -> 3x3 conv on [64(H),2,64(W)] via 18 fused scalar_tensor_tensor taps
  -> sigmoid -> PE ones-broadcast into PSUM -> out = (x*s)*sig in place
HBM traffic is the minimum: read x once, write out once (64 MiB/core).

The walrus build here allows one sync wait per instruction, no ext-isa
gpsimd ops, no 2-input Pool ops, and no partition-base mismatch between
SBUF operands; waitsplit.split_multi_waits handles the wait limit and
the kernel avoids the rest.
"""
import numpy as np
from contextlib import ExitStack

import concourse.bass as bass
import concourse.mybir as mybir
from concourse.tile import TileContext
from concourse.bass_utils import run_bass_kernel_spmd

F32 = mybir.dt.float32
BF16 = mybir.dt.bfloat16
ALU = mybir.AluOpType
ACTF = mybir.ActivationFunctionType
AX = mybir.AxisListType

B, C, H, W = 32, 512, 64, 64
NCORES = 8
BPC = B // NCORES          # samples per core
HWF = H * W                # 4096
NCH = C // 128             # 4 channel chunks of 128
Cr = C // 8                # 64
EPS = 1e-12

_cache = {}


def _split_multi_waits(nc):
    import bass_rust
    fn = nc.m.functions[0]
    n_split = 0
    uid = 0
    for bb in list(fn.blocks):
        insts = bb.instructions
        out = []
        changed = False
        for ins in insts:
            si = ins.sync_info
            waits = list(si.on_wait) if si and si.on_wait else []
            if len(waits) > 1:
                changed = True
                for w in waits[:-1]:
                    nop = bass_rust.InstNoOp(name=f"wsplit_{uid}")
                    uid += 1
                    nop.engine = ins.engine
                    nop.sync_info = bass_rust.SyncInfo(on_wait=[w], on_update=[])
                    nc.register_instruction(nop, overwrite=True)
                    out.append(nop)
                    n_split += 1
                ins.sync_info = bass_rust.SyncInfo(
                    on_wait=waits[-1:], on_update=list(si.on_update or []))
            out.append(ins)
        if changed:
            bb.instructions = out
    return n_split


def _build_nc():
    nc = bass.Bass("TRN2", debug=False)

    x_ext = nc.declare_dram_parameter("x", [BPC, C, H, W], F32, isOutput=False)
    w1_ext = nc.declare_dram_parameter("w1", [Cr, C], F32, isOutput=False)       # W1m
    w1t_ext = nc.declare_dram_parameter("w1t", [128, NCH, Cr], F32, isOutput=False)  # W1m.T chunks
    b1_ext = nc.declare_dram_parameter("b1", [Cr, 1], F32, isOutput=False)
    u1t_ext = nc.declare_dram_parameter("u1t", [Cr, 1], F32, isOutput=False)
    w2_ext = nc.declare_dram_parameter("w2", [128, NCH, Cr], F32, isOutput=False)    # W2m chunks
    w2t_ext = nc.declare_dram_parameter("w2t", [Cr, NCH, 128], F32, isOutput=False)  # W2m.T chunks
    b2_ext = nc.declare_dram_parameter("b2", [128, NCH], F32, isOutput=False)
    u2t_ext = nc.declare_dram_parameter("u2t", [128, NCH], F32, isOutput=False)
    w3_ext = nc.declare_dram_parameter("w3", [1, 18], F32, isOutput=False)
    b3_ext = nc.declare_dram_parameter("b3", [1, 1], F32, isOutput=False)
    u3_ext = nc.declare_dram_parameter("u3", [1, 1], F32, isOutput=False)
    id_ext = nc.declare_dram_parameter("ident", [128, 128], F32, isOutput=False)
    out_ext = nc.declare_dram_parameter("out", [BPC, C, H, W], F32, isOutput=True)
    sig_scr = nc.dram_tensor("sig_scratch", [BPC, H, W], F32)

    # DRAM views: [128 part, chunk, hw]
    x_v = [x_ext[b].rearrange("c h w -> c (h w)").rearrange("(g p) f -> p g f", p=128)
           for b in range(BPC)]
    o_v = [out_ext[b].rearrange("c h w -> c (h w)").rearrange("(g p) f -> p g f", p=128)
           for b in range(BPC)]

    with TileContext(nc) as tc, ExitStack() as ctx:
        const = ctx.enter_context(tc.tile_pool(name="const", bufs=1))
        xpool = ctx.enter_context(tc.tile_pool(name="x", bufs=4))
        mid = ctx.enter_context(tc.tile_pool(name="mid", bufs=3))
        csr = ctx.enter_context(tc.tile_pool(name="csr", bufs=1))
        stats = ctx.enter_context(tc.tile_pool(name="stats", bufs=3))
        convp_ = ctx.enter_context(tc.tile_pool(name="conv", bufs=3))

        # ---------------- setup: load params ----------------
        w1sb = const.tile([Cr, C], F32)
        nc.sync.dma_start(out=w1sb[:], in_=w1_ext[:])
        w1t = const.tile([128, NCH, Cr], F32)
        nc.sync.dma_start(out=w1t[:], in_=w1t_ext[:])
        b1t = const.tile([Cr, 1], F32)
        nc.sync.dma_start(out=b1t[:], in_=b1_ext[:])
        u1t = const.tile([Cr, 1], F32)
        nc.sync.dma_start(out=u1t[:], in_=u1t_ext[:])
        w2sb = const.tile([128, NCH, Cr], F32)
        nc.sync.dma_start(out=w2sb[:], in_=w2_ext[:])
        w2t = const.tile([Cr, NCH, 128], F32)
        nc.sync.dma_start(out=w2t[:], in_=w2t_ext[:])
        b2c = const.tile([128, NCH], F32)
        nc.sync.dma_start(out=b2c[:], in_=b2_ext[:])
        u2t = const.tile([128, NCH], F32)
        nc.sync.dma_start(out=u2t[:], in_=u2t_ext[:])
        w3sb = const.tile([1, 18], F32)
        nc.sync.dma_start(out=w3sb[:], in_=w3_ext[:])
        b3sb = const.tile([1, 1], F32)
        nc.sync.dma_start(out=b3sb[:], in_=b3_ext[:])
        u3sb = const.tile([1, 1], F32)
        nc.sync.dma_start(out=u3sb[:], in_=u3_ext[:])
        ident = const.tile([128, 128], F32)
        nc.sync.dma_start(out=ident[:], in_=id_ext[:])

        ones = const.tile([128, 1], F32)
        nc.vector.memset(ones[:], 1.0)
        onesr = const.tile([1, 128], F32)
        nc.vector.memset(onesr[:], 1.0)
        b2x2 = const.tile([128, NCH], F32)
        nc.vector.tensor_scalar(b2x2[:], b2c[:], 2.0, None, ALU.mult)

        taps64 = const.tile([H, 18], F32)
        b3rep = const.tile([H, 1], F32)
        scal128 = const.tile([128, 3], F32)

        # ---------------- setup: spectral norms ----------------
        with tc.tile_pool(name="pssn", bufs=2, space="PSUM") as pssn, \
             tc.tile_pool(name="snsb", bufs=1) as snsb:
            # --- sv1 for w1 ([Cr, C]); sv = ||q|| / ||r||, r = Wm.T u, q = Wm r ---
            r1t_ps = pssn.tile([128, NCH], F32, tag="big")
            for c_ in range(NCH):
                nc.tensor.matmul(r1t_ps[:, c_:c_ + 1], w1sb[:, c_ * 128:(c_ + 1) * 128],
                                 u1t[:], start=True, stop=True)
            r1t = snsb.tile([128, NCH], F32)
            nc.scalar.copy(r1t[:], r1t_ps[:])
            r1sq = snsb.tile([128, NCH], F32)
            r1sqa = snsb.tile([128, 1], F32)
            nc.scalar.activation(out=r1sq[:], in_=r1t_ps[:], func=ACTF.Square,
                                 accum_out=r1sqa[:])
            n1_ps = pssn.tile([1, 1], F32, tag="dot")
            nc.tensor.matmul(n1_ps[:], r1sqa[:], ones[:], start=True, stop=True)
            n1 = snsb.tile([1, 1], F32)
            nc.scalar.activation(out=n1[:], in_=n1_ps[:], func=ACTF.Sqrt)
            nc.vector.tensor_scalar(n1[:], n1[:], EPS, None, ALU.max)
            rinv1 = snsb.tile([1, 1], F32)
            nc.vector.reciprocal(rinv1[:], n1[:])

            q1_ps = pssn.tile([Cr, 1], F32, tag="big")
            for c_ in range(NCH):
                nc.tensor.matmul(q1_ps[:], w1t[:, c_, :], r1t[:, c_:c_ + 1],
                                 start=(c_ == 0), stop=(c_ == NCH - 1))
            q1sq = snsb.tile([Cr, 1], F32)
            nc.scalar.activation(out=q1sq[:], in_=q1_ps[:], func=ACTF.Square)
            nq1_ps = pssn.tile([1, 1], F32, tag="dot")
            nc.tensor.matmul(nq1_ps[:], q1sq[:], ones[0:Cr, :], start=True, stop=True)
            sv1 = snsb.tile([1, 1], F32)
            nc.scalar.activation(out=sv1[:], in_=nq1_ps[:], func=ACTF.Sqrt)
            nc.vector.tensor_tensor(sv1[:], sv1[:], rinv1[:], ALU.mult)
            inv1 = snsb.tile([1, 1], F32)
            nc.vector.reciprocal(inv1[:], sv1[:])
            csavg = snsb.tile([1, 1], F32)
            nc.vector.tensor_scalar(csavg[:], inv1[:], 1.0 / HWF, None, ALU.mult)

            # --- sv2 for w2 ([C, Cr]) ---
            r2t_ps = pssn.tile([Cr, 1], F32, tag="big")
            for c_ in range(NCH):
                nc.tensor.matmul(r2t_ps[:], w2sb[:, c_, :], u2t[:, c_:c_ + 1],
                                 start=(c_ == 0), stop=(c_ == NCH - 1))
            r2t = snsb.tile([Cr, 1], F32)
            nc.scalar.copy(r2t[:], r2t_ps[:])
            r2sq = snsb.tile([Cr, 1], F32)
            nc.scalar.activation(out=r2sq[:], in_=r2t_ps[:], func=ACTF.Square)
            n2_ps = pssn.tile([1, 1], F32, tag="dot")
            nc.tensor.matmul(n2_ps[:], r2sq[:], ones[0:Cr, :], start=True, stop=True)
            n2 = snsb.tile([1, 1], F32)
            nc.scalar.activation(out=n2[:], in_=n2_ps[:], func=ACTF.Sqrt)
            nc.vector.tensor_scalar(n2[:], n2[:], EPS, None, ALU.max)
            rinv2 = snsb.tile([1, 1], F32)
            nc.vector.reciprocal(rinv2[:], n2[:])

            q2_ps = pssn.tile([128, NCH], F32, tag="big")
            for c_ in range(NCH):
                nc.tensor.matmul(q2_ps[:, c_:c_ + 1], w2t[:, c_, :], r2t[:],
                                 start=True, stop=True)
            q2sq = snsb.tile([128, NCH], F32)
            q2sqa = snsb.tile([128, 1], F32)
            nc.scalar.activation(out=q2sq[:], in_=q2_ps[:], func=ACTF.Square,
                                 accum_out=q2sqa[:])
            nq2_ps = pssn.tile([1, 1], F32, tag="dot")
            nc.tensor.matmul(nq2_ps[:], q2sqa[:], ones[:], start=True, stop=True)
            sv2 = snsb.tile([1, 1], F32)
            nc.scalar.activation(out=sv2[:], in_=nq2_ps[:], func=ACTF.Sqrt)
            nc.vector.tensor_tensor(sv2[:], sv2[:], rinv2[:], ALU.mult)
            inv2 = snsb.tile([1, 1], F32)
            nc.vector.reciprocal(inv2[:], sv2[:])

            # --- sv3 for w3 ([1, 18]) ---
            v3 = snsb.tile([1, 18], F32)
            nc.vector.tensor_scalar(v3[:], w3sb[:], u3sb[0:1, 0:1], None, ALU.mult)
            v3sq = snsb.tile([1, 18], F32)
            n3sq = snsb.tile([1, 1], F32)
            nc.scalar.activation(out=v3sq[:], in_=v3[:], func=ACTF.Square,
                                 accum_out=n3sq[:])
            n3 = snsb.tile([1, 1], F32)
            nc.scalar.activation(out=n3[:], in_=n3sq[:], func=ACTF.Sqrt)
            nc.vector.tensor_scalar(n3[:], n3[:], EPS, None, ALU.max)
            rinv3 = snsb.tile([1, 1], F32)
            nc.vector.reciprocal(rinv3[:], n3[:])
            t3 = snsb.tile([1, 18], F32)
            nc.vector.tensor_tensor(t3[:], v3[:], w3sb[:], ALU.mult)
            t3s = snsb.tile([1, 1], F32)
            nc.vector.tensor_reduce(out=t3s[:], in_=t3[:], axis=AX.X, op=ALU.add)
            nc.vector.tensor_tensor(t3s[:], t3s[:], rinv3[:], ALU.mult)
            sv3 = snsb.tile([1, 1], F32)
            nc.scalar.activation(out=sv3[:], in_=t3s[:], func=ACTF.Abs)
            inv3 = snsb.tile([1, 1], F32)
            nc.vector.reciprocal(inv3[:], sv3[:])

            # --- fold scales: taps (w3/sv3, ch0 also /C), replicated scalars ---
            tapr = snsb.tile([1, 18], F32)
            nc.vector.tensor_scalar(tapr[:], w3sb[:], inv3[0:1, 0:1], None, ALU.mult)
            nc.vector.tensor_scalar(tapr[0:1, 0:9], tapr[0:1, 0:9], 1.0 / C, None,
                                    ALU.mult)
            scalr = snsb.tile([1, 3], F32)
            nc.vector.tensor_copy(scalr[0:1, 0:1], csavg[:])
            nc.vector.tensor_copy(scalr[0:1, 1:2], inv1[:])
            nc.vector.tensor_copy(scalr[0:1, 2:3], inv2[:])

            # replicate across partitions via PE ones-broadcast
            bc_ps = pssn.tile([128, 32], F32, tag="big")
            nc.tensor.matmul(bc_ps[0:H, 0:18], onesr[0:1, 0:H], tapr[:],
                             start=True, stop=True)
            nc.tensor.matmul(bc_ps[0:H, 18:19], onesr[0:1, 0:H], b3sb[:],
                             start=True, stop=True)
            nc.tensor.matmul(bc_ps[:, 19:22], onesr[:], scalr[:],
                             start=True, stop=True)
            nc.scalar.copy(taps64[:], bc_ps[0:H, 0:18])
            nc.scalar.copy(b3rep[:], bc_ps[0:H, 18:19])
            nc.scalar.copy(scal128[:], bc_ps[:, 19:22])

        # main-loop PSUM pools (created after the setup pool is closed so the
        # stack allocator reuses its banks; total = 2+2+2+2 = 8 banks)
        psmlp = ctx.enter_context(tc.tile_pool(name="psmlp", bufs=1, space="PSUM"))
        pscs = ctx.enter_context(tc.tile_pool(name="pscs", bufs=1, space="PSUM"))
        pstp = ctx.enter_context(tc.tile_pool(name="pstp", bufs=3, space="PSUM"))
        xb16p = ctx.enter_context(tc.tile_pool(name="xb16", bufs=2))

        # ---------------- main loop over samples ----------------
        for b in range(BPC):
            xt = []
            for hf in range(2):
                xh = xpool.tile([128, 2, HWF], F32, tag="x")
                nc.sync.dma_start(out=xh[:], in_=x_v[b][:, 2 * hf:2 * hf + 2, :])
                xt.append(xh)
            xv = [xt[cc // 2][:, cc % 2, :] for cc in range(NCH)]

            # t: cmax-chain accumulator; also scratch dest for ACT sum-pool
            t = mid.tile([128, HWF], F32, tag="mid")

            avgs = stats.tile([128, NCH], F32, tag="avg")
            mxs = stats.tile([128, NCH], F32, tag="mx")
            for cc in range(NCH):
                nc.scalar.activation(out=t[:], in_=xv[cc], func=ACTF.Copy,
                                     accum_out=avgs[:, cc:cc + 1])
                nc.vector.tensor_reduce(out=mxs[:, cc:cc + 1], in_=xv[cc],
                                        axis=AX.X, op=ALU.max)

            # channel-attention MLP -> s (chunked [128, NCH])
            h_ps = psmlp.tile([Cr, 2], F32, tag="mlp")
            for c_ in range(NCH):
                nc.tensor.matmul(h_ps[:, 0:1], w1t[:, c_, :], avgs[:, c_:c_ + 1],
                                 start=(c_ == 0), stop=(c_ == NCH - 1))
            for c_ in range(NCH):
                nc.tensor.matmul(h_ps[:, 1:2], w1t[:, c_, :], mxs[:, c_:c_ + 1],
                                 start=(c_ == 0), stop=(c_ == NCH - 1))
            h_sb = stats.tile([Cr, 2], F32, tag="hsb")
            nc.scalar.activation(out=h_sb[:, 0:1], in_=h_ps[:, 0:1], func=ACTF.Relu,
                                 bias=b1t[:], scale=scal128[0:Cr, 0:1])
            nc.scalar.activation(out=h_sb[:, 1:2], in_=h_ps[:, 1:2], func=ACTF.Relu,
                                 bias=b1t[:], scale=scal128[0:Cr, 1:2])
            hs = stats.tile([Cr, 1], F32, tag="hs")
            nc.vector.tensor_tensor(hs[:], h_sb[:, 0:1], h_sb[:, 1:2], ALU.add)
            a_ps = psmlp.tile([128, NCH], F32, tag="mlp")
            for c_ in range(NCH):
                nc.tensor.matmul(a_ps[:, c_:c_ + 1], w2t[:, c_, :], hs[:],
                                 start=True, stop=True)
            s_t = stats.tile([128, NCH], F32, tag="st")
            for c_ in range(NCH):
                nc.scalar.activation(out=s_t[:, c_:c_ + 1], in_=a_ps[:, c_:c_ + 1],
                                     func=ACTF.Sigmoid, bias=b2x2[:, c_:c_ + 1],
                                     scale=scal128[:, 2:3])

            # channel max of s*x: ACT init + DVE stt chain -> t
            nc.scalar.activation(out=t[:], in_=xv[0], func=ACTF.Copy,
                                 scale=s_t[:, 0:1])
            for cc in range(1, NCH):
                nc.vector.scalar_tensor_tensor(out=t[:], in0=xv[cc],
                                               scalar=s_t[:, cc:cc + 1], in1=t[:],
                                               op0=ALU.mult, op1=ALU.max)
            # partition reduce via PE transposes: cmaxT[p, m] = max_c t[c, 128m+p]
            cmaxT = stats.tile([128, 32], F32, tag="cmaxT")
            for g in range(8):
                ps = pstp.tile([128, 4, 128], F32, tag="tp")
                for mm in range(4):
                    m = 4 * g + mm
                    nc.tensor.transpose(ps[:, mm, :], t[:, 128 * m:128 * (m + 1)],
                                        ident[:])
                nc.vector.tensor_reduce(out=cmaxT[:, 4 * g:4 * (g + 1)], in_=ps[:],
                                        axis=AX.X, op=ALU.max)
            t2 = pstp.tile([32, 128], F32, tag="tp")
            nc.tensor.transpose(t2[:], cmaxT[:], ident[:])
            t2sb = convp_.tile([32, 128], F32, tag="t2sb")
            nc.scalar.copy(t2sb[:], t2[:])
            conv_in = convp_.tile([H, 2, W], F32, tag="cin")
            nc.sync.dma_start(out=conv_in[:, 1, :],
                              in_=t2sb[:].rearrange("q (r w) -> q r w", r=2))

            # channel sum of s*x via PE in bf16 (stat path tolerates bf16;
            # gpsimd converts x chunks just-in-time), evac + reshape per chunk
            s16 = stats.tile([128, NCH], BF16, tag="s16")
            nc.vector.tensor_copy(s16[:], s_t[:])
            CSW = 2048
            RPC = CSW // W
            for jh in range(HWF // CSW):
                ps = pscs.tile([1, CSW], F32, tag="cs")
                for c_ in range(NCH):
                    xb = xb16p.tile([128, CSW], BF16, tag="xb")
                    if c_ % 2 == 0:
                        nc.vector.tensor_copy(xb[:], xv[c_][:, CSW * jh:CSW * (jh + 1)])
                    else:
                        nc.scalar.copy(xb[:], xv[c_][:, CSW * jh:CSW * (jh + 1)])
                    for q in range(CSW // 512):
                        nc.tensor.matmul(ps[0:1, 512 * q:512 * (q + 1)],
                                         s16[:, c_:c_ + 1], xb[:, 512 * q:512 * (q + 1)],
                                         start=(c_ == 0), stop=(c_ == NCH - 1))
                crow = csr.tile([1, CSW], F32, tag="csrow")
                nc.scalar.copy(crow[:], ps[:])
                nc.sync.dma_start(
                    out=conv_in[RPC * jh:RPC * (jh + 1), 0, :],
                    in_=crow[0:1, :].rearrange("p (h w) -> p h w", h=RPC))

            # 3x3 conv, zero pad: H-shifts via DMA copies, W-shifts via free offsets
            convm = convp_.tile([H, 2, W], F32, tag="cm")   # [r] = conv_in[r+1]
            convp = convp_.tile([H, 2, W], F32, tag="cp")   # [r] = conv_in[r-1]
            nc.gpsimd.memset(convm[:], 0.0)
            nc.gpsimd.memset(convp[:], 0.0)
            nc.sync.dma_start(out=convm[0:H - 1, :, :], in_=conv_in[1:H, :, :])
            nc.sync.dma_start(out=convp[1:H, :, :], in_=conv_in[0:H - 1, :, :])
            y64 = convp_.tile([H, W], F32, tag="y")
            srcs = {0: convp, 1: conv_in, 2: convm}
            nc.vector.tensor_scalar(y64[:], conv_in[:, 0, :], taps64[:, 4:5], None,
                                    ALU.mult)
            for ch in range(2):
                for kh in range(3):
                    src = srcs[kh]
                    for kw in range(3):
                        j = ch * 9 + kh * 3 + kw
                        if j == 4:
                            continue
                        if kw == 1:
                            o_sl, i_sl = slice(0, W), slice(0, W)
                        elif kw == 2:
                            o_sl, i_sl = slice(0, W - 1), slice(1, W)
                        else:
                            o_sl, i_sl = slice(1, W), slice(0, W - 1)
                        nc.vector.scalar_tensor_tensor(
                            out=y64[:, o_sl], in0=src[:, ch, i_sl],
                            scalar=taps64[:, j:j + 1], in1=y64[:, o_sl],
                            op0=ALU.mult, op1=ALU.add)
            nc.scalar.activation(out=y64[:], in_=y64[:], func=ACTF.Sigmoid,
                                 bias=b3rep[:])

            # sigmoid broadcast over channels via DRAM bounce:
            # y64 -> DRAM scratch -> partition-broadcast read -> sigB
            nc.sync.dma_start(out=sig_scr[b], in_=y64[:])
            flat = sig_scr[b].rearrange("h w -> (h w)")
            bcast_ap = bass.AP(tensor=flat.tensor, offset=flat.offset,
                               ap=[[0, 128]] + list(flat.ap))
            sigB = mid.tile([128, HWF], F32, tag="mid")
            nc.sync.dma_start(out=sigB[:], in_=bcast_ap)
            for cc in range(NCH):
                nc.vector.scalar_tensor_tensor(
                    out=xv[cc], in0=xv[cc],
                    scalar=s_t[:, cc:cc + 1], in1=sigB[:],
                    op0=ALU.mult, op1=ALU.mult)
            for hf in range(2):
                nc.sync.dma_start(out=o_v[b][:, 2 * hf:2 * hf + 2, :], in_=xt[hf][:])

    _split_multi_waits(nc)
    return nc


def _get_nc():
    if "nc" not in _cache:
        _cache["nc"] = _build_nc()
    return _cache["nc"]


def _prep_in_maps(inputs):
    f = lambda a: np.ascontiguousarray(np.asarray(a, dtype=np.float32))
    x = f(inputs["x"])
    w1 = f(inputs["w1"]).reshape(Cr, C)
    w2 = f(inputs["w2"]).reshape(C, Cr)
    w3 = f(inputs["w3"]).reshape(1, 18)
    common = {
        "w1": w1,
        "w1t": np.ascontiguousarray(w1.T.reshape(NCH, 128, Cr).transpose(1, 0, 2)),
        "b1": f(inputs["b1"]).reshape(Cr, 1),
        "u1t": f(inputs["u1"]).reshape(Cr, 1),
        "w2": np.ascontiguousarray(w2.reshape(NCH, 128, Cr).transpose(1, 0, 2)),
        "w2t": np.ascontiguousarray(w2.T.reshape(Cr, NCH, 128)),
        "b2": np.ascontiguousarray(f(inputs["b2"]).reshape(NCH, 128).T),
        "u2t": np.ascontiguousarray(f(inputs["u2"]).reshape(NCH, 128).T),
        "w3": w3,
        "b3": f(inputs["b3"]).reshape(1, 1),
        "u3": f(inputs["u3"]).reshape(1, 1),
        "ident": np.eye(128, dtype=np.float32),
    }
    return [dict(common, x=np.ascontiguousarray(x[k * BPC:(k + 1) * BPC]))
            for k in range(NCORES)]


def run(inputs, trace=False, **kw):
    nc = _get_nc()
    in_maps = _prep_in_maps(inputs)
    res = run_bass_kernel_spmd(nc, in_maps, list(range(NCORES)), trace=trace, **kw)
    out = np.concatenate([res.results[k]["out"] for k in range(NCORES)], axis=0)
    return out, res


def kernel(**inputs) -> np.ndarray:
    out, _ = run(inputs)
    return out
